# revision 1
# baseline (speedup 1.0000x reference)
"""Trainium2 Bass kernel: single-head causal attention (v2).

Reference computation (B=4, S=4096, E=1024, L=64):
    Q = x @ Wq + bq ; K = x @ Wk + bk ; V = x @ Wv + bv
    scores = Q @ K^T / sqrt(64), causal-masked, softmax over kv
    out = attn @ V

Sharding: 2 cores per batch, interleaved-parity q-tile ownership (16 of
32 q-tiles each), full kv per core.  One SPMD graph for all 8 cores;
parity differences are carried in input data only (kv column
permutation + two small mask tensors).

v2 structural changes over the first working version:
  - exp batching: score chunks are packed into PSUM tiles of up to
    1536 columns and exp'd with ONE ACTIVATE per batch (fewer ACT
    instructions + semaphores; ACT was the pacing engine).
  - minimal masking: only diagonal chunks need a (static, shared)
    triangular 128x128 multiply; odd-window-position chunks need a
    per-core all-0/all-1 column-block multiply (tensor_scalar).
    Everything else is exactly-width valid.
  - slots processed in order 1,2,3 with slot 0's small attention woven
    into slot 3's stretch as PE filler; epilogues chained at the end.
  - projections for quarter g+1 emitted interleaved between slot-g
    attention batches so the PE never idles (keeps HAM clock warm).
"""

import math
from contextlib import ExitStack

import ml_dtypes
import numpy as np

import concourse.bass as bass
import concourse.mybir as mybir
import concourse.tile as tile
from concourse import bacc
from concourse.bass_utils import run_bass_kernel_spmd

B, S, E, L = 4, 4096, 1024, 64
P = 128
NCORES = 8
NQUART = 4
SEGW = 512
QW = 1024
SCALE = 1.0 / math.sqrt(L)

BF16 = mybir.dt.bfloat16
F32 = mybir.dt.float32
NPBF16 = ml_dtypes.bfloat16

# width schedule for window position k = chunk - 8g (shared by both
# parities; odd positions include one extra block that parity 0 zeroes
# via pmask, even positions have the triangular diagonal at their
# leftmost block)
WSCHED = [512, 512, 384, 384, 256, 256, 128, 128]
BATCH_MAX = 1536  # 3 PSUM banks per batch tile
VSTR = 68  # vch per-chunk stride (4-byte-aligned: 68*2B = 136B)


def _perm_tile(g, k, p):
    """Global kv tile stored at permuted chunk position 8g+k for parity p."""
    return 8 * g + (p if k % 2 == 0 else 1 - p) + (k - k % 2)


def _own_tiles(p):
    """Global q-tile indices owned by parity p, in stored (packed) order."""
    return [8 * g + p + 2 * b for g in range(NQUART) for b in range(4)]


def _chunk_width(g, c):
    k = c - 8 * g
    return SEGW if k < 0 else WSCHED[k]


def _chunk_loc(c):
    """Storage of chunk position c under the [own|other] half layout:
    returns (segment, block).  Even positions (own-parity tiles) live in
    the quarter's first 512-col half, odd in the second."""
    j = c % 8
    return 2 * (c // 8) + (j % 2), j // 2


# boundary chunks emitted same-parity-adjacent (even positions, then
# odd) so bank-sharing chunks are always serialized on the same PE row
# group; full chunks alternate parity in exclusive banks and pair up
# concurrently on disjoint row groups.
BOUNDARY_ORDER = [0, 2, 6, 4, 1, 3, 7, 5]


def _batches(g):
    """Pack chunks of slot g into batches of total width <= BATCH_MAX,
    bank-aligned (a matmul PSUM write must not cross a 512-col bank).
    Returns lists of (chunk, width, offset)."""
    order = list(range(8 * g)) + [8 * g + k for k in BOUNDARY_ORDER]
    out = []
    cur = []
    w_acc = 0
    for c in order:
        w = _chunk_width(g, c)
        if w_acc // SEGW != (w_acc + w - 1) // SEGW:
            w_acc = -(-w_acc // SEGW) * SEGW
        if w_acc + w > BATCH_MAX:
            out.append(cur)
            cur = []
            w_acc = 0
        cur.append((c, w, w_acc))
        w_acc += w
    if cur:
        out.append(cur)
    return out


# packed-constant column offsets
CWS = 8 * P            # swapped [Wv|Wk] weights (odd segments)
CWQ = CWS + 8 * P      # wq starts after both weight sets
CID = CWQ + 8 * L      # identity (bf16)
CDM = CID + P          # diagonal mask
CPB_W = CDM + P
# f32 pack: bkv | bkv_swapped | bq | bv | idnf | pmask
CBQ = 2
CBV = 3
CIDF = CBV + L
CPM = CIDF + P
CPF_W = CPM + 1

_GRAPH_CACHE = {}


def _build_graph():
    if "nc" in _GRAPH_CACHE:
        return _GRAPH_CACHE["nc"]
    nc = bacc.Bacc()

    xt = nc.declare_dram_parameter("xt", [8, P, S], BF16, isOutput=False)
    # packed constants, host-laid-out in final SBUF layout:
    # bf16 pack: wkv (8x128) | wq (8x64) | idn (128) | dmask (128)
    # f32 pack:  bkv (1) | bq (1) | bv (64) | idnf (128) | pmask (1)
    cpb = nc.declare_dram_parameter("cpb", [P, CPB_W], BF16, isOutput=False)
    cpf = nc.declare_dram_parameter("cpf", [P, CPF_W], F32, isOutput=False)
    out = nc.declare_dram_parameter("out", [4 * (L + 1), SEGW], F32,
                                    isOutput=True)

    Exp = mybir.ActivationFunctionType.Exp
    Add = mybir.AluOpType.add
    Mult = mybir.AluOpType.mult

    with ExitStack() as ctx:
        tc = ctx.enter_context(tile.TileContext(nc))
        singles = ctx.enter_context(tc.tile_pool(name="singles", bufs=1))
        xpool = ctx.enter_context(tc.tile_pool(name="xq", bufs=1))
        kvpool = ctx.enter_context(tc.tile_pool(name="kv", bufs=1))
        vpool = ctx.enter_context(tc.tile_pool(name="v", bufs=1))
        qpool = ctx.enter_context(tc.tile_pool(name="q", bufs=1))
        epool = ctx.enter_context(tc.tile_pool(name="expT", bufs=10))
        otpool = ctx.enter_context(tc.tile_pool(name="oT", bufs=2))
        opool = ctx.enter_context(tc.tile_pool(name="osb", bufs=3))
        # PSUM: pss 2x3 banks + po 1 + psB 1 = 8
        psS = ctx.enter_context(tc.tile_pool(name="psS", bufs=2, space="PSUM"))
        psO = ctx.enter_context(tc.tile_pool(name="psO", bufs=1, space="PSUM"))
        psB = ctx.enter_context(tc.tile_pool(name="psB", bufs=1, space="PSUM"))

        # ACT table warmup: dependency-free scratch exp carries the
        # table-set load with zero sync waits
        scratch = singles.tile([P, 32], F32, tag="scratch")
        nc.scalar.activation(scratch[:], scratch[:], Exp)

        # PE clock warmup: dense dummy matmuls during the initial DMA
        # phase keep the HAM at K=8 so real matmuls start at 2.4 GHz
        warm = singles.tile([P, SEGW], BF16, tag="warm")
        nc.vector.memset(warm[:], 0.0)
        for i in range(14):
            pw = psS.tile([P, BATCH_MAX], F32, tag="mm")
            nc.tensor.matmul(pw[:, 0:SEGW], warm[:, 0:P], warm[:],
                             start=True, stop=True, skip_group_check=True)

        # --- batched loads; quarter 0 first (it gates the pipeline) ---
        cpb_s = singles.tile([P, CPB_W], BF16, tag="cpb")
        cpf_s = singles.tile([P, CPF_W], F32, tag="cpf")
        xq = []
        for g in range(NQUART):
            xq_g = xpool.tile([P, 8 * QW], BF16, tag=f"x{g}")
            xq.append(xq_g)

        def load_half(g, h):
            """Contiguous load of one 512-col half of quarter g (the own-
            parity half is h=0, other-parity h=1 under the host layout)."""
            c0 = g * QW + h * SEGW
            nc.sync.dma_start(
                out=xq[g][:].rearrange(
                    "p (e n) -> p e n", n=QW)[:, :, h * SEGW:(h + 1) * SEGW],
                in_=xt[:, :, c0:c0 + SEGW].rearrange("e p n -> p e n"))

        load_half(0, 0)
        nc.sync.dma_start(out=cpb_s[:], in_=cpb[:])
        nc.sync.dma_start(out=cpf_s[:], in_=cpf[:])

        def load_full(g):
            nc.sync.dma_start(
                out=xq[g][:].rearrange("p (e n) -> p e n", n=QW),
                in_=xt[:, :, g * QW:(g + 1) * QW].rearrange("e p n -> p e n"))

        load_half(0, 1)
        load_half(1, 0)
        load_half(1, 1)
        load_full(2)
        load_full(3)

        kvt = {}   # per 512-col segment: [128, 512] bf16 ([KT; VT] rows)
        # all V chunks in one tile: chunk c at cols 65c..65c+64, ones col at 65c+64
        vch = vpool.tile([P, 32 * VSTR], BF16, tag="vch")
        nc.vector.memset(
            vch[:].rearrange("p (c w) -> p c w", w=VSTR)[:, :, L:L + 1], 1.0)
        qt = {}    # per slot: [64, 512] bf16 (own q tiles, QT layout)

        def emit_kv_proj(s):
            """KV projection for 512-col segment s.  Even segments use
            [Wk|Wv] so K^T lands at partitions 0:64; odd segments use the
            swapped [Wv|Wk] so K^T lands at partitions 64:128.  Scores for
            a chunk then read K at base partition (position%2)*64, which
            auto-derives the PE row-group and lets adjacent score matmuls
            run concurrently on disjoint array halves."""
            g, h = s // 2, s % 2
            w0 = 0 if h == 0 else CWS
            ps = psS.tile([P, BATCH_MAX], F32, tag="mm")
            for e in range(8):
                nc.tensor.matmul(
                    ps[:, 0:SEGW], cpb_s[:, w0 + e * P:w0 + (e + 1) * P],
                    xq[g][:, e * QW + h * SEGW: e * QW + (h + 1) * SEGW],
                    start=(e == 0), stop=(e == 7), skip_group_check=True)
            kt = kvpool.tile([P, SEGW], BF16, tag=f"kv{s}")
            nc.vector.tensor_scalar_add(kt[:], ps[:, 0:SEGW],
                                        cpf_s[:, h:h + 1])
            kvt[s] = kt

        def emit_v_trans(s):
            """V transposes for segment s (deferred so the transpose
            weight-loads have long-satisfied deps and pull ahead under
            the running attention stream).  V rows are 64:128 (even
            segment) / 0:64 (odd)."""
            g, h = s // 2, s % 2
            kt = kvt[s]
            v0 = L if h == 0 else 0
            pv = psB.tile([P, 4 * L], BF16, tag="tp")
            for cc in range(4):
                nc.tensor.transpose(
                    pv[:, cc * L:(cc + 1) * L],
                    kt[v0:v0 + L, cc * P:(cc + 1) * P],
                    cpb_s[v0:v0 + L, CID:CID + L])
            for cc in range(4):
                c = s * 4 + cc
                nc.vector.tensor_copy(
                    vch[:, c * VSTR:c * VSTR + L],
                    pv[:, cc * L:(cc + 1) * L])

        def emit_kv_seg(s):
            emit_kv_proj(s)
            emit_v_trans(s)

        def emit_q(g):
            """Q projection for slot g, computed twice via concurrent
            column-group matmuls so qt ends up replicated at partitions
            0:64 AND 64:128 (the odd-row-group score matmuls need their
            rhs at base partition 64)."""
            ps = psS.tile([P, BATCH_MAX], F32, tag="mm")
            for e in range(8):
                for half in range(2):
                    nc.tensor.matmul(
                        ps[half * L:(half + 1) * L, 0:SEGW],
                        cpb_s[:, CWQ + e * L:CWQ + (e + 1) * L],
                        xq[g][:, e * QW:e * QW + SEGW],
                        start=(e == 0), stop=(e == 7),
                        skip_group_check=True)
            q = qpool.tile([P, SEGW], BF16, tag=f"q{g}")
            nc.vector.tensor_scalar_add(
                q[:], ps[:, 0:SEGW],
                cpf_s[:, CBQ:CBQ + 1])
            qt[g] = q

        # ---- filler machinery: projection work woven between batches ----
        filler = []  # list of zero-arg emit thunks

        def drain_filler(n):
            for _ in range(min(n, len(filler))):
                filler.pop(0)()

        # ---- attention ----
        def emit_batch_scores(g, batch, etag="e"):
            """Score matmuls + one exp + masks for one batch. Returns et."""
            pss = psS.tile([P, BATCH_MAX], F32, tag="mm")
            W = 0
            for c, w, off in batch:
                seg, blk = _chunk_loc(c)
                rh = (c % 2) * L
                nc.tensor.matmul(
                    pss[:, off:off + w],
                    kvt[seg][rh:rh + L, blk * P:(blk + 1) * P],
                    qt[g][rh:rh + L, SEGW - w:SEGW],
                    start=True, stop=True, skip_group_check=True)
                W = off + w
            et = epool.tile([P, BATCH_MAX], BF16, tag=etag)
            nc.scalar.activation(et[:, 0:W], pss[:, 0:W], Exp)
            # masks: only the leftmost block of window-position chunks
            for c, w, off in batch:
                k = c - 8 * g
                if k < 0:
                    continue
                if k % 2 == 0:
                    # diagonal chunk: static triangular mask on leftmost block
                    nc.vector.tensor_tensor(
                        et[:, off:off + P], et[:, off:off + P],
                        cpb_s[:, CDM:CDM + P], Mult)
                else:
                    # parity pad block: multiply by per-core 0/1 scalar
                    nc.vector.tensor_scalar_mul(
                        et[:, off:off + P], et[:, off:off + P], cpf_s[:, CPM:CPM + 1])
            return et

        def emit_batch_av(g, batch, et, po, is_first, is_last):
            for i, (c, w, off) in enumerate(batch):
                seg, blk = _chunk_loc(c)
                vc = seg * 4 + blk
                nc.tensor.matmul(
                    po[:, SEGW - w:SEGW],
                    vch[:, vc * VSTR:vc * VSTR + L + 1],
                    et[:, off:off + w],
                    start=(is_first and i == 0),
                    stop=(is_last and i == len(batch) - 1),
                    skip_group_check=True)

        def emit_epilogue(g, po, mul_on_act=False):
            # ship the raw accumulator + denominator row; the host does
            # the divide and bias during unsharding (it already reorders)
            ot = otpool.tile([L + 1, SEGW], F32, tag="ot")
            nc.vector.tensor_copy(ot[:], po[:])
            nc.sync.dma_start(
                out=out[g * (L + 1):(g + 1) * (L + 1), :], in_=ot[:])

        def emit_slot(g, po, inject=None):
            """Emit slot g's attention with 1-batch AV lag and filler weave.

            inject: optional dict {batch_index: thunk} run between batches
            (used to weave slot 0's score/exp work into slot 3).
            """
            batches = _batches(g)
            pend = []  # [(batch, et, is_first), ...] with lag 3
            for i, batch in enumerate(batches):
                et = emit_batch_scores(g, batch)
                if len(pend) == 3:
                    b0 = pend.pop(0)
                    emit_batch_av(g, b0[0], b0[1], po, b0[2], False)
                drain_filler(1)
                if inject and i in inject:
                    inject[i]()
                pend.append((batch, et, i == 0))
            for j, b0 in enumerate(pend):
                emit_batch_av(g, b0[0], b0[1], po, b0[2],
                              j == len(pend) - 1)

        # ---- schedule ----
        # quarter-0 projections + Q0 up front (PE work during DMA); slot 0
        # needs only quarter 0 so its attention starts earliest
        emit_kv_seg(0)
        emit_q(0)
        emit_kv_seg(1)

        # filler for slot 0's stretch: quarter 1 projections + Q1
        filler.append(lambda: emit_q(1))
        filler.append(lambda: emit_kv_proj(2))
        filler.append(lambda: emit_v_trans(2))
        filler.append(lambda: emit_kv_proj(3))
        filler.append(lambda: emit_v_trans(3))

        po0 = psO.tile([L + 1, SEGW], F32, tag="po")
        emit_slot(0, po0)
        emit_epilogue(0, po0)

        # filler for slot 1's stretch: quarter 2 projections + Q2
        filler.append(lambda: emit_q(2))
        filler.append(lambda: emit_kv_proj(4))
        filler.append(lambda: emit_v_trans(4))
        filler.append(lambda: emit_kv_proj(5))
        filler.append(lambda: emit_v_trans(5))

        po1 = psO.tile([L + 1, SEGW], F32, tag="po")
        emit_slot(1, po1)
        emit_epilogue(1, po1)

        # filler for slot 2's stretch: quarter 3 projections + Q3
        filler.append(lambda: emit_q(3))

        po2 = psO.tile([L + 1, SEGW], F32, tag="po")
        emit_slot(2, po2)
        emit_epilogue(2, po2)

        po3 = psO.tile([L + 1, SEGW], F32, tag="po")
        filler.append(lambda: emit_kv_proj(6))
        filler.append(lambda: emit_v_trans(6))
        filler.append(lambda: emit_kv_proj(7))
        filler.append(lambda: emit_v_trans(7))
        emit_slot(3, po3)
        emit_epilogue(3, po3, mul_on_act=True)

    nc.compile()
    _GRAPH_CACHE["nc"] = nc
    return nc


def kernel(x, Wq, Wk, Wv, bq, bk, bv, mask):
    x = np.asarray(x, dtype=np.float32)
    Wq = np.asarray(Wq, dtype=np.float32)
    Wk = np.asarray(Wk, dtype=np.float32)
    Wv = np.asarray(Wv, dtype=np.float32)
    bq_ = np.asarray(bq, dtype=np.float32)
    bk_ = np.asarray(bk, dtype=np.float32)
    bv_ = np.asarray(bv, dtype=np.float32)

    nc = _build_graph()

    wkv_np = np.concatenate([Wk, Wv], axis=1).reshape(8, P, P)
    wq_np = (Wq * SCALE).reshape(8, P, L)
    # bf16 pack: [wkv per-e | wq per-e | identity | dmask], already in
    # on-chip [p, cols] layout
    wvk_np = np.concatenate([Wv, Wk], axis=1).reshape(8, P, P)
    cpb_np = np.zeros((P, CPB_W), dtype=NPBF16)
    for e in range(8):
        cpb_np[:, e * P:(e + 1) * P] = wkv_np[e].astype(NPBF16)
        cpb_np[:, CWS + e * P:CWS + (e + 1) * P] = wvk_np[e].astype(NPBF16)
        cpb_np[:, CWQ + e * L:CWQ + (e + 1) * L] = wq_np[e].astype(NPBF16)
    id_np = np.zeros((P, P), dtype=NPBF16)
    id_np[0:L, 0:L] = np.eye(L)
    id_np[L:P, 0:L] = np.eye(L)
    cpb_np[:, CID:CID + P] = id_np
    i = np.arange(P)[:, None]
    u = np.arange(P)[None, :]
    cpb_np[:, CDM:CDM + P] = (i <= u).astype(NPBF16)
    # f32 pack: [bkv | bq | bv | idnf | pmask]
    cpf_base = np.zeros((P, CPF_W), dtype=np.float32)
    cpf_base[:, 0] = np.concatenate([bk_, np.zeros(L, np.float32)])
    cpf_base[:, 1] = np.concatenate([np.zeros(L, np.float32), bk_])
    cpf_base[:, CBQ] = np.concatenate([bq_, bq_]) * SCALE
    cpf_base[:, CBV:CBV + L] = np.tile(bv_[None, :], (P, 1))
    cpf_base[:, CIDF:CIDF + P] = np.eye(P, dtype=np.float32)

    in_maps = []
    for core in range(NCORES):
        b, p = core // 2, core % 2
        tiles = [8 * g + par + 2 * bb
                 for g in range(NQUART) for par in (p, 1 - p)
                 for bb in range(4)]
        colperm = np.concatenate([np.arange(t * P, t * P + P) for t in tiles])
        xt_np = np.ascontiguousarray(
            x[b].T[:, colperm]).reshape(8, P, S).astype(NPBF16)
        cpf_np = cpf_base.copy()
        cpf_np[:, CPM] = 0.0 if p == 0 else 1.0
        in_maps.append({"xt": xt_np, "cpb": cpb_np, "cpf": cpf_np})

    # out = softmax(scores) @ V + bv is mathematically bounded by
    # max|V| + |bv|; retry on the rare transient where an execution
    # returns garbage
    for attempt in range(3):
        res = run_bass_kernel_spmd(nc, in_maps, core_ids=list(range(NCORES)))
        out_full = np.empty((B, S, L), dtype=np.float32)
        for core in range(NCORES):
            b, p = core // 2, core % 2
            o = res.results[core]["out"].reshape(4, L + 1, SEGW)
            vals = o[:, 0:L, :]                      # [slot, l, q]
            den = o[:, L, :]                         # [slot, q]
            norm = vals / den[:, None, :]            # [slot, l, q]
            for g in range(NQUART):
                for bb in range(4):
                    t = 8 * g + p + 2 * bb
                    out_full[b, t * P:(t + 1) * P, :] = (
                        norm[g, :, bb * P:(bb + 1) * P].T + bv_)
        if np.isfinite(out_full).all() and np.abs(out_full).max() < 100.0:
            break
    return out_full



# revision 6
# speedup vs baseline: 1.0191x; 1.0191x over previous
"""Trainium2 Bass kernel: single-head causal attention (v3).

Reference computation (B=4, S=4096, E=1024, L=64):
    Q = x @ Wq + bq ; K = x @ Wk + bk ; V = x @ Wv + bv
    scores = Q @ K^T / sqrt(64), causal-masked, softmax over kv
    out = attn @ V

Sharding: 2 cores per batch, interleaved-parity q-tile ownership (16 of
32 q-tiles each), full kv per core.  One SPMD graph for all 8 cores;
parity differences are carried in input data only (kv column
permutation + two small mask tensors).

v3 structural changes over v2:
  - startup: cpb/cpf DMAs issued first, first x half-load split in two
    pieces so the seg-0 kv projection can start as soon as the first
    four e-chunks land; warmup count retuned to bridge exactly.
  - all filler projections (kv proj + q proj) write a dedicated psum
    bank (psB) instead of rotating through the score banks, removing
    the slot-boundary WAR stall against the batch exp reads.
  - V transposes emitted as cross-segment pairs on disjoint PE row
    groups (even seg V rows 64:128 = h64, odd seg rows 0:64 = h0) so
    adjacent transposes run concurrently.
  - slot 3: last score batch split so the final exp is short, and the
    epilogue is flushed progressively (cols 0:256 as soon as their
    accumulation completes, remainder at the end) to shorten the tail.
"""

import math
from contextlib import ExitStack

import ml_dtypes
import numpy as np

import concourse.bass as bass
import concourse.mybir as mybir
import concourse.tile as tile
from concourse import bacc
from concourse.bass_utils import run_bass_kernel_spmd

B, S, E, L = 4, 4096, 1024, 64
P = 128
NCORES = 8
NQUART = 4
SEGW = 512
QW = 1024
SCALE = 1.0 / math.sqrt(L)

BF16 = mybir.dt.bfloat16
F32 = mybir.dt.float32
NPBF16 = ml_dtypes.bfloat16

# width schedule for window position k = chunk - 8g (shared by both
# parities; odd positions include one extra block that parity 0 zeroes
# via pmask, even positions have the triangular diagonal at their
# leftmost block)
WSCHED = [512, 512, 384, 384, 256, 256, 128, 128]
BATCH_MAX = 1536  # 3 PSUM banks per batch tile
VSTR = 68  # vch per-chunk stride (4-byte-aligned: 68*2B = 136B)
WARM_N = 11  # PE warmup matmuls bridging the initial DMA window


def _perm_tile(g, k, p):
    """Global kv tile stored at permuted chunk position 8g+k for parity p."""
    return 8 * g + (p if k % 2 == 0 else 1 - p) + (k - k % 2)


def _own_tiles(p):
    """Global q-tile indices owned by parity p, in stored (packed) order."""
    return [8 * g + p + 2 * b for g in range(NQUART) for b in range(4)]


def _chunk_width(g, c):
    k = c - 8 * g
    return SEGW if k < 0 else WSCHED[k]


def _chunk_loc(c):
    """Storage of chunk position c under the [own|other] half layout:
    returns (segment, block).  Even positions (own-parity tiles) live in
    the quarter's first 512-col half, odd in the second."""
    j = c % 8
    return 2 * (c // 8) + (j % 2), j // 2


# boundary chunks emitted same-parity-adjacent (even positions, then
# odd) so bank-sharing chunks are always serialized on the same PE row
# group; full chunks alternate parity in exclusive banks and pair up
# concurrently on disjoint row groups.
BOUNDARY_ORDER = [0, 2, 6, 4, 1, 3, 7, 5]


def _pack(chunks, widths):
    """Assign bank-aligned offsets to a chunk list, capped at BATCH_MAX.
    Returns list of batches [(c, w, off), ...]."""
    out = []
    cur = []
    w_acc = 0
    for c in chunks:
        w = widths[c]
        if w_acc // SEGW != (w_acc + w - 1) // SEGW:
            w_acc = -(-w_acc // SEGW) * SEGW
        if w_acc + w > BATCH_MAX:
            out.append(cur)
            cur = []
            w_acc = 0
        cur.append((c, w, w_acc))
        w_acc += w
    if cur:
        out.append(cur)
    return out


def _batches(g, tail_max=None):
    """Pack chunks of slot g into batches of total width <= BATCH_MAX,
    bank-aligned (a matmul PSUM write must not cross a 512-col bank).
    With tail_max, the final batch is re-split so the trailing piece is
    <= tail_max wide (keeps the last exp short)."""
    order = list(range(8 * g)) + [8 * g + k for k in BOUNDARY_ORDER]
    widths = {c: _chunk_width(g, c) for c in order}
    out = _pack(order, widths)
    if tail_max and len(out[-1]) > 1:
        tail = [c for c, w, off in out.pop()]
        acc = 0
        k = len(tail)
        while k > 1 and acc + widths[tail[k - 1]] <= tail_max:
            acc += widths[tail[k - 1]]
            k -= 1
        if 0 < k < len(tail):
            out.extend(_pack(tail[:k], widths))
            out.extend(_pack(tail[k:], widths))
        else:
            out.extend(_pack(tail, widths))
    return out


# packed-constant column offsets
CWS = 8 * P            # swapped [Wv|Wk] weights (odd segments)
CWQ = CWS + 8 * P      # wq starts after both weight sets
CID = CWQ + 8 * L      # identity (bf16)
CDM = CID + P          # diagonal mask
CPB_W = CDM + P
# f32 pack: bkv | bkv_swapped | bq | bv | idnf | pmask
CBQ = 2
CBV = 3
CIDF = CBV + L
CPM = CIDF + P
CPF_W = CPM + 1

import os
V3_SPLIT_DMA = os.environ.get("V3_SPLIT_DMA", "1") == "1"
V3_PSB = os.environ.get("V3_PSB", "1") == "1"
V3_VTPAIR = os.environ.get("V3_VTPAIR", "0") == "1"
V3_PROG = os.environ.get("V3_PROG", "1") == "1"

_GRAPH_CACHE = {}


def _build_graph():
    if "nc" in _GRAPH_CACHE:
        return _GRAPH_CACHE["nc"]
    nc = bacc.Bacc()

    xt = nc.declare_dram_parameter("xt", [8, P, S], BF16, isOutput=False)
    # packed constants, host-laid-out in final SBUF layout:
    # bf16 pack: wkv (8x128) | wq (8x64) | idn (128) | dmask (128)
    # f32 pack:  bkv (1) | bq (1) | bv (64) | idnf (128) | pmask (1)
    cpb = nc.declare_dram_parameter("cpb", [P, CPB_W], BF16, isOutput=False)
    cpf = nc.declare_dram_parameter("cpf", [P, CPF_W], F32, isOutput=False)
    out = nc.declare_dram_parameter("out", [4 * (L + 1), SEGW], F32,
                                    isOutput=True)

    Exp = mybir.ActivationFunctionType.Exp
    Add = mybir.AluOpType.add
    Mult = mybir.AluOpType.mult

    with ExitStack() as ctx:
        tc = ctx.enter_context(tile.TileContext(nc))
        singles = ctx.enter_context(tc.tile_pool(name="singles", bufs=1))
        xpool = ctx.enter_context(tc.tile_pool(name="xq", bufs=1))
        kvpool = ctx.enter_context(tc.tile_pool(name="kv", bufs=1))
        vpool = ctx.enter_context(tc.tile_pool(name="v", bufs=1))
        qpool = ctx.enter_context(tc.tile_pool(name="q", bufs=1))
        epool = ctx.enter_context(tc.tile_pool(name="expT", bufs=10))
        otpool = ctx.enter_context(tc.tile_pool(name="oT", bufs=2))
        opool = ctx.enter_context(tc.tile_pool(name="osb", bufs=3))
        # PSUM: psS 2x3 banks + psO 1 + psB 1 = 8
        psS = ctx.enter_context(tc.tile_pool(name="psS", bufs=2, space="PSUM"))
        psO = ctx.enter_context(tc.tile_pool(name="psO", bufs=1, space="PSUM"))
        psB = ctx.enter_context(tc.tile_pool(name="psB", bufs=1, space="PSUM"))

        # ACT table warmup: dependency-free scratch exp carries the
        # table-set load with zero sync waits
        scratch = singles.tile([P, 32], F32, tag="scratch")
        nc.scalar.activation(scratch[:], scratch[:], Exp)

        # PE clock warmup: dense dummy matmuls during the initial DMA
        # phase keep the HAM at K=8 so real matmuls start at 2.4 GHz
        warm = singles.tile([P, SEGW], BF16, tag="warm")
        nc.vector.memset(warm[:], 0.0)
        for i in range(WARM_N):
            pw = psS.tile([P, BATCH_MAX], F32, tag="mm")
            nc.tensor.matmul(pw[:, 0:SEGW], warm[:, 0:P], warm[:],
                             start=True, stop=True, skip_group_check=True)

        # --- batched loads; constants first (they gate the first
        # projection's weights), then quarter 0 piece-wise ---
        cpb_s = singles.tile([P, CPB_W], BF16, tag="cpb")
        cpf_s = singles.tile([P, CPF_W], F32, tag="cpf")
        nc.sync.dma_start(out=cpb_s[:], in_=cpb[:])
        nc.sync.dma_start(out=cpf_s[:], in_=cpf[:])

        xq = []
        for g in range(NQUART):
            xq_g = xpool.tile([P, 8 * QW], BF16, tag=f"x{g}")
            xq.append(xq_g)

        def load_piece(g, h, e0, e1):
            """Load e-chunks [e0:e1) of one 512-col half of quarter g."""
            c0 = g * QW + h * SEGW
            nc.sync.dma_start(
                out=xq[g][:].rearrange(
                    "p (e n) -> p e n", n=QW)[:, e0:e1, h * SEGW:(h + 1) * SEGW],
                in_=xt[e0:e1, :, c0:c0 + SEGW].rearrange("e p n -> p e n"))

        def load_full(g):
            nc.sync.dma_start(
                out=xq[g][:].rearrange("p (e n) -> p e n", n=QW),
                in_=xt[:, :, g * QW:(g + 1) * QW].rearrange("e p n -> p e n"))

        if V3_SPLIT_DMA:
            load_piece(0, 0, 0, 4)
            load_piece(0, 0, 4, 8)
        else:
            load_piece(0, 0, 0, 8)
        load_piece(0, 1, 0, 8)
        load_piece(1, 0, 0, 8)
        load_piece(1, 1, 0, 8)
        load_full(2)
        load_full(3)

        kvt = {}   # per 512-col segment: [128, 512] bf16 ([KT; VT] rows)
        # all V chunks in one tile: chunk c at cols 65c..65c+64, ones col at 65c+64
        vch = vpool.tile([P, 32 * VSTR], BF16, tag="vch")
        nc.vector.memset(
            vch[:].rearrange("p (c w) -> p c w", w=VSTR)[:, :, L:L + 1], 1.0)
        qt = {}    # per slot: [64, 512] bf16 (own q tiles, QT layout)

        def emit_kv_proj(s, pool):
            """KV projection for 512-col segment s.  Even segments use
            [Wk|Wv] so K^T lands at partitions 0:64; odd segments use the
            swapped [Wv|Wk] so K^T lands at partitions 64:128.  Scores for
            a chunk then read K at base partition (position%2)*64, which
            auto-derives the PE row-group and lets adjacent score matmuls
            run concurrently on disjoint array halves."""
            g, h = s // 2, s % 2
            w0 = 0 if h == 0 else CWS
            if pool is psS:
                ps = pool.tile([P, BATCH_MAX], F32, tag="mm")
            else:
                ps = pool.tile([P, SEGW], F32, tag="pb")
            for e in range(8):
                nc.tensor.matmul(
                    ps[:, 0:SEGW], cpb_s[:, w0 + e * P:w0 + (e + 1) * P],
                    xq[g][:, e * QW + h * SEGW: e * QW + (h + 1) * SEGW],
                    start=(e == 0), stop=(e == 7), skip_group_check=True)
            kt = kvpool.tile([P, SEGW], BF16, tag=f"kv{s}")
            nc.vector.tensor_scalar_add(kt[:], ps[:, 0:SEGW],
                                        cpf_s[:, h:h + 1])
            kvt[s] = kt

        def emit_vt_pair(sa, sb):
            """V transposes for segments sa (even -> V rows 64:128, row
            group h64) and sb (odd -> rows 0:64, h0), interleaved so each
            adjacent transpose pair runs concurrently on disjoint PE row
            halves."""
            if V3_VTPAIR:
                pv = psB.tile([P, 8 * L], BF16, tag="pb")
                for cc in range(4):
                    for k, s in ((0, sa), (1, sb)):
                        v0 = L if s % 2 == 0 else 0
                        nc.tensor.transpose(
                            pv[:, (2 * cc + k) * L:(2 * cc + k + 1) * L],
                            kvt[s][v0:v0 + L, cc * P:(cc + 1) * P],
                            cpb_s[v0:v0 + L, CID:CID + L])
                for cc in range(4):
                    for k, s in ((0, sa), (1, sb)):
                        c = s * 4 + cc
                        nc.vector.tensor_copy(
                            vch[:, c * VSTR:c * VSTR + L],
                            pv[:, (2 * cc + k) * L:(2 * cc + k + 1) * L])
            else:
                for s in (sa, sb):
                    v0 = L if s % 2 == 0 else 0
                    pv = psB.tile([P, 4 * L], BF16, tag="pb")
                    for cc in range(4):
                        nc.tensor.transpose(
                            pv[:, cc * L:(cc + 1) * L],
                            kvt[s][v0:v0 + L, cc * P:(cc + 1) * P],
                            cpb_s[v0:v0 + L, CID:CID + L])
                    for cc in range(4):
                        c = s * 4 + cc
                        nc.vector.tensor_copy(
                            vch[:, c * VSTR:c * VSTR + L],
                            pv[:, cc * L:(cc + 1) * L])

        def emit_q(g, pool):
            """Q projection for slot g, computed twice via concurrent
            column-group matmuls so qt ends up replicated at partitions
            0:64 AND 64:128 (the odd-row-group score matmuls need their
            rhs at base partition 64)."""
            if pool is psS:
                ps = pool.tile([P, BATCH_MAX], F32, tag="mm")
            else:
                ps = pool.tile([P, SEGW], F32, tag="pb")
            for e in range(8):
                for half in range(2):
                    nc.tensor.matmul(
                        ps[half * L:(half + 1) * L, 0:SEGW],
                        cpb_s[:, CWQ + e * L:CWQ + (e + 1) * L],
                        xq[g][:, e * QW:e * QW + SEGW],
                        start=(e == 0), stop=(e == 7),
                        skip_group_check=True)
            q = qpool.tile([P, SEGW], BF16, tag=f"q{g}")
            nc.vector.tensor_scalar_add(
                q[:], ps[:, 0:SEGW],
                cpf_s[:, CBQ:CBQ + 1])
            qt[g] = q

        # ---- filler machinery: projection work woven between batches ----
        filler = []  # list of zero-arg emit thunks

        def drain_filler(n):
            for _ in range(min(n, len(filler))):
                filler.pop(0)()

        # ---- attention ----
        def emit_batch_scores(g, batch, etag="e"):
            """Score matmuls + one exp + masks for one batch. Returns et."""
            pss = psS.tile([P, BATCH_MAX], F32, tag="mm")
            W = 0
            for c, w, off in batch:
                seg, blk = _chunk_loc(c)
                rh = (c % 2) * L
                nc.tensor.matmul(
                    pss[:, off:off + w],
                    kvt[seg][rh:rh + L, blk * P:(blk + 1) * P],
                    qt[g][rh:rh + L, SEGW - w:SEGW],
                    start=True, stop=True, skip_group_check=True)
                W = off + w
            et = epool.tile([P, BATCH_MAX], BF16, tag=etag)
            nc.scalar.activation(et[:, 0:W], pss[:, 0:W], Exp)
            # masks: only the leftmost block of window-position chunks
            for c, w, off in batch:
                k = c - 8 * g
                if k < 0:
                    continue
                if k % 2 == 0:
                    # diagonal chunk: static triangular mask on leftmost block
                    nc.vector.tensor_tensor(
                        et[:, off:off + P], et[:, off:off + P],
                        cpb_s[:, CDM:CDM + P], Mult)
                else:
                    # parity pad block: multiply by per-core 0/1 scalar
                    nc.vector.tensor_scalar_mul(
                        et[:, off:off + P], et[:, off:off + P], cpf_s[:, CPM:CPM + 1])
            return et

        def emit_batch_av(g, batch, et, po, is_first, is_last):
            for i, (c, w, off) in enumerate(batch):
                seg, blk = _chunk_loc(c)
                vc = seg * 4 + blk
                nc.tensor.matmul(
                    po[:, SEGW - w:SEGW],
                    vch[:, vc * VSTR:vc * VSTR + L + 1],
                    et[:, off:off + w],
                    start=(is_first and i == 0),
                    stop=(is_last and i == len(batch) - 1),
                    skip_group_check=True)

        def emit_epilogue(g, po, c0=0, c1=SEGW):
            # ship the raw accumulator + denominator row; the host does
            # the divide and bias during unsharding (it already reorders)
            ot = otpool.tile([L + 1, SEGW], F32, tag="ot")
            nc.vector.tensor_copy(ot[:, c0:c1], po[:, c0:c1])
            nc.sync.dma_start(
                out=out[g * (L + 1):(g + 1) * (L + 1), c0:c1],
                in_=ot[:, c0:c1])

        def emit_slot(g, po, prog=False):
            """Emit slot g's attention with 3-batch AV lag and filler weave.

            prog: progressively flush epilogue columns whose accumulation
            has completed (used for the final slot to shorten the tail).
            The caller must still emit the final epilogue piece.
            """
            batches = _batches(g, tail_max=SEGW if prog else None)

            flushed = 0
            j01 = -1
            if prog:
                def touches(batch, blo, bhi):
                    return any(SEGW - w < 128 * bhi for c, w, off in batch)
                j01 = max(j for j, b in enumerate(batches) if touches(b, 0, 2))

            n_av = 0

            def av_emitted(j, po):
                nonlocal flushed
                if prog and j == j01 and j01 < len(batches) - 1:
                    emit_epilogue(g, po, 0, 256)
                    flushed = 256

            pend = []  # [(batch, et, is_first), ...] with lag 3
            for i, batch in enumerate(batches):
                et = emit_batch_scores(g, batch)
                if len(pend) == 3:
                    b0 = pend.pop(0)
                    emit_batch_av(g, b0[0], b0[1], po, b0[2], False)
                    av_emitted(n_av, po)
                    n_av += 1
                drain_filler(1)
                pend.append((batch, et, i == 0))
            for j, b0 in enumerate(pend):
                emit_batch_av(g, b0[0], b0[1], po, b0[2],
                              j == len(pend) - 1)
                av_emitted(n_av, po)
                n_av += 1
            return flushed

        # ---- schedule ----
        # quarter-0 projections + Q0 up front (PE work during DMA); slot 0
        # needs only quarter 0 so its attention starts earliest.  Startup
        # projections rotate through the score banks (no exp WAR yet);
        # everything drained as filler later uses the dedicated psB bank.
        emit_kv_proj(0, psS)
        emit_q(0, psS)
        emit_kv_proj(1, psS)
        emit_vt_pair(0, 1)

        # filler for slot 0's stretch: quarter 1 projections + Q1
        fpool = psB if V3_PSB else psS
        filler.append(lambda: emit_q(1, fpool))
        filler.append(lambda: emit_kv_proj(2, fpool))
        filler.append(lambda: emit_kv_proj(3, fpool))
        filler.append(lambda: emit_vt_pair(2, 3))

        po0 = psO.tile([L + 1, SEGW], F32, tag="po")
        emit_slot(0, po0)
        emit_epilogue(0, po0)

        # filler for slot 1's stretch: quarter 2 projections + Q2
        filler.append(lambda: emit_q(2, fpool))
        filler.append(lambda: emit_kv_proj(4, fpool))
        filler.append(lambda: emit_kv_proj(5, fpool))
        filler.append(lambda: emit_vt_pair(4, 5))

        po1 = psO.tile([L + 1, SEGW], F32, tag="po")
        emit_slot(1, po1)
        emit_epilogue(1, po1)

        # filler for slot 2's stretch: quarter 3 projections + Q3
        filler.append(lambda: emit_q(3, fpool))

        po2 = psO.tile([L + 1, SEGW], F32, tag="po")
        emit_slot(2, po2)
        emit_epilogue(2, po2)

        po3 = psO.tile([L + 1, SEGW], F32, tag="po")
        filler.append(lambda: emit_kv_proj(6, fpool))
        filler.append(lambda: emit_kv_proj(7, fpool))
        filler.append(lambda: emit_vt_pair(6, 7))
        flushed = emit_slot(3, po3, prog=V3_PROG)
        emit_epilogue(3, po3, flushed, SEGW)

    nc.compile()
    _GRAPH_CACHE["nc"] = nc
    return nc


def kernel(x, Wq, Wk, Wv, bq, bk, bv, mask):
    x = np.asarray(x, dtype=np.float32)
    Wq = np.asarray(Wq, dtype=np.float32)
    Wk = np.asarray(Wk, dtype=np.float32)
    Wv = np.asarray(Wv, dtype=np.float32)
    bq_ = np.asarray(bq, dtype=np.float32)
    bk_ = np.asarray(bk, dtype=np.float32)
    bv_ = np.asarray(bv, dtype=np.float32)

    nc = _build_graph()

    wkv_np = np.concatenate([Wk, Wv], axis=1).reshape(8, P, P)
    wq_np = (Wq * SCALE).reshape(8, P, L)
    # bf16 pack: [wkv per-e | wq per-e | identity | dmask], already in
    # on-chip [p, cols] layout
    wvk_np = np.concatenate([Wv, Wk], axis=1).reshape(8, P, P)
    cpb_np = np.zeros((P, CPB_W), dtype=NPBF16)
    for e in range(8):
        cpb_np[:, e * P:(e + 1) * P] = wkv_np[e].astype(NPBF16)
        cpb_np[:, CWS + e * P:CWS + (e + 1) * P] = wvk_np[e].astype(NPBF16)
        cpb_np[:, CWQ + e * L:CWQ + (e + 1) * L] = wq_np[e].astype(NPBF16)
    id_np = np.zeros((P, P), dtype=NPBF16)
    id_np[0:L, 0:L] = np.eye(L)
    id_np[L:P, 0:L] = np.eye(L)
    cpb_np[:, CID:CID + P] = id_np
    i = np.arange(P)[:, None]
    u = np.arange(P)[None, :]
    cpb_np[:, CDM:CDM + P] = (i <= u).astype(NPBF16)
    # f32 pack: [bkv | bq | bv | idnf | pmask]
    cpf_base = np.zeros((P, CPF_W), dtype=np.float32)
    cpf_base[:, 0] = np.concatenate([bk_, np.zeros(L, np.float32)])
    cpf_base[:, 1] = np.concatenate([np.zeros(L, np.float32), bk_])
    cpf_base[:, CBQ] = np.concatenate([bq_, bq_]) * SCALE
    cpf_base[:, CBV:CBV + L] = np.tile(bv_[None, :], (P, 1))
    cpf_base[:, CIDF:CIDF + P] = np.eye(P, dtype=np.float32)

    in_maps = []
    for core in range(NCORES):
        b, p = core // 2, core % 2
        tiles = [8 * g + par + 2 * bb
                 for g in range(NQUART) for par in (p, 1 - p)
                 for bb in range(4)]
        colperm = np.concatenate([np.arange(t * P, t * P + P) for t in tiles])
        xt_np = np.ascontiguousarray(
            x[b].T[:, colperm]).reshape(8, P, S).astype(NPBF16)
        cpf_np = cpf_base.copy()
        cpf_np[:, CPM] = 0.0 if p == 0 else 1.0
        in_maps.append({"xt": xt_np, "cpb": cpb_np, "cpf": cpf_np})

    # out = softmax(scores) @ V + bv is mathematically bounded by
    # max|V| + |bv|; retry on the rare transient where an execution
    # returns garbage
    for attempt in range(3):
        res = run_bass_kernel_spmd(nc, in_maps, core_ids=list(range(NCORES)))
        out_full = np.empty((B, S, L), dtype=np.float32)
        for core in range(NCORES):
            b, p = core // 2, core % 2
            o = res.results[core]["out"].reshape(4, L + 1, SEGW)
            vals = o[:, 0:L, :]                      # [slot, l, q]
            den = o[:, L, :]                         # [slot, q]
            norm = vals / den[:, None, :]            # [slot, l, q]
            for g in range(NQUART):
                for bb in range(4):
                    t = 8 * g + p + 2 * bb
                    out_full[b, t * P:(t + 1) * P, :] = (
                        norm[g, :, bb * P:(bb + 1) * P].T + bv_)
        if np.isfinite(out_full).all() and np.abs(out_full).max() < 100.0:
            break
    return out_full
